# revision 1
# baseline (speedup 1.0000x reference)
"""AUGRU (attention-update GRU) cell for Trainium2, 8 NeuronCores.

Strategy: pure data parallelism over the batch. Each of the 8 cores gets a
1024-row shard of input_x / input_h / attention_score and a replica of the six
512x512 weight matrices, computes its shard of

    r = sigmoid(x@Wx_r + b_r + h@Wh_r)
    u = sigmoid(x@Wx_u + b_u + h@Wh_u)
    c = tanh(x@Wx_h + b_h + r*(h@Wh_h))
    out = (1 - att*u)*h + att*u*c

and the host gathers the 8 output shards. No collectives are needed.

Kernel details (per core, per 128-row batch tile):
  - x/h tiles are transposed on the TensorEngine (via identity matmul) so the
    contraction dim lands on SBUF partitions; results are rounded to float32r.
  - The six matmuls run in float32r (full PE rate, ~1.5e-4 rel err) as
    PSUM-accumulation groups over four 128-wide k-chunks.
  - Biases enter as a rank-1 ones-vector matmul at the head of each group
    (skipped entirely when every bias is exactly zero, as in this problem).
  - sigmoid/tanh run on the ScalarEngine, the interpolation on the
    VectorEngine with a fused (u*att)*d scalar_tensor_tensor op.
"""
import numpy as np
import concourse.bass as bass
import concourse.mybir as mybir
from concourse import bacc, masks
from concourse.tile import TileContext
from concourse.bass_utils import run_bass_kernel_spmd

F32 = mybir.dt.float32
F32R = mybir.dt.float32r
AF = mybir.ActivationFunctionType
ALU = mybir.AluOpType

N_CORES = 8
B = 8192
D = 512                  # D_IN == UNITS
BT = 128                 # rows per batch tile (SBUF partition count)
BS = B // N_CORES        # 1024 rows per core
NB = BS // BT            # 8 batch tiles per core
KC = D // 128            # 4 contraction chunks
GATES = ("r", "u", "h")

# Weight-DMA issue order matches matmul group order (r, hh, h, u) so the first
# batch tile's groups unblock as early as possible during the initial load.
W_ORDER = ("xr", "hr", "hh", "xh", "xu", "hu")
GROUP_ORDER = ("r", "hh", "h", "u")


def build(has_bias: bool, loop: int = 0, staggered: bool = True):
    """Build + compile the per-core program. loop>0 wraps the body in a
    hardware For_i loop (used only for wall-clock timing harnesses)."""
    nc = bacc.Bacc("TRN2", target_bir_lowering=False, debug=False,
                   num_devices=N_CORES)

    x_d = nc.dram_tensor("x", [BS, D], F32, kind="ExternalInput")
    h_d = nc.dram_tensor("h", [BS, D], F32, kind="ExternalInput")
    att_d = nc.dram_tensor("att", [BS, 1], F32, kind="ExternalInput")
    w_d, b_d = {}, {}
    for g in GATES:
        w_d["x" + g] = nc.dram_tensor(f"Wx_{g}", [D, D], F32, kind="ExternalInput")
        w_d["h" + g] = nc.dram_tensor(f"Wh_{g}", [D, D], F32, kind="ExternalInput")
        b_d[g] = nc.dram_tensor(f"b_{g}", [D], F32, kind="ExternalInput")
    out_d = nc.dram_tensor("out", [BS, D], F32, kind="ExternalOutput")

    def load_w(wpool):
        w_sb = {}
        for wk in W_ORDER:
            t = wpool.tile([128, KC * D], F32R, tag=f"w_{wk}", name=f"w_{wk}")
            for j in range(KC):
                nc.sync.dma_start(out=t[:, j * D:(j + 1) * D],
                                  in_=w_d[wk][j * 128:(j + 1) * 128, :].bitcast(F32R))
            w_sb[wk] = t
        b_sb = {}
        if has_bias:
            for g in GATES:
                t = wpool.tile([1, D], F32, tag=f"b_{g}", name=f"bias_{g}")
                nc.sync.dma_start(out=t[:], in_=b_d[g][None, :])
                b_sb[g] = t
        return w_sb, b_sb

    def body(w_sb, b_sb, ident, ones, xpool, tppool, gppool, spool):
        # attention scores for all 8 tiles in one DMA: [128, 8], col i = tile i
        att_all = xpool.tile([BT, NB], F32, tag="att_all", name="att_all")
        nc.scalar.dma_start(out=att_all[:],
                            in_=att_d[:].rearrange("(t p) o -> p (t o)", p=BT))
        for i in range(NB):
            row = slice(i * BT, (i + 1) * BT)
            x_sb = xpool.tile([BT, D], F32, tag="x", name="x_sb")
            nc.scalar.dma_start(out=x_sb[:], in_=x_d[row, :])
            h_sb = xpool.tile([BT, D], F32, tag="h", name="h_sb")
            nc.scalar.dma_start(out=h_sb[:], in_=h_d[row, :])
            att_sb = att_all[:, i:i + 1]

            # transpose x/h k-chunks on PE; DVE copies round fp32 -> f32r
            pt_x = tppool.tile([128, D], F32, tag="pt_x", name="pt_x")
            pt_h = tppool.tile([128, D], F32, tag="pt_h", name="pt_h")
            for j in range(KC):
                cs = slice(j * 128, (j + 1) * 128)
                nc.tensor.transpose(pt_x[:, cs], x_sb[:, cs], ident[:])
                nc.tensor.transpose(pt_h[:, cs], h_sb[:, cs], ident[:])
            xT = spool.tile([128, D], F32R, tag="xT", name="xT")
            nc.vector.tensor_copy(xT[:], pt_x[:])
            hT = spool.tile([128, D], F32R, tag="hT", name="hT")
            nc.vector.tensor_copy(hT[:], pt_h[:])

            ps = {}
            for g in ("r", "u", "h", "hh"):
                ps[g] = gppool.tile([BT, D], F32, tag=f"ps_{g}", name=f"ps_{g}")

            parts = {
                "r": [(xT, "xr"), (hT, "hr")],
                "u": [(xT, "xu"), (hT, "hu")],
                "h": [(xT, "xh")],
                "hh": [(hT, "hh")],
            }
            for gname in GROUP_ORDER:
                psum = ps[gname]
                first = True
                if has_bias and gname != "hh":
                    nc.tensor.matmul(psum[:], ones[:], b_sb[gname][:],
                                     start=True, stop=False)
                    first = False
                total = len(parts[gname]) * KC
                k = 0
                for (lhs, wk) in parts[gname]:
                    for j in range(KC):
                        cs = slice(j * 128, (j + 1) * 128)
                        k += 1
                        nc.tensor.matmul(psum[:], lhs[:, cs],
                                         w_sb[wk][:, j * D:(j + 1) * D],
                                         start=(first and k == 1),
                                         stop=(k == total))

            r_sb = spool.tile([BT, D], F32, tag="r", name="r_sb")
            nc.scalar.activation(r_sb[:], ps["r"][:], AF.Sigmoid)
            u_sb = spool.tile([BT, D], F32, tag="u", name="u_sb")
            nc.scalar.activation(u_sb[:], ps["u"][:], AF.Sigmoid)
            t_sb = spool.tile([BT, D], F32, tag="t", name="t_sb")
            nc.vector.tensor_mul(t_sb[:], r_sb[:], ps["hh"][:])
            t2_sb = spool.tile([BT, D], F32, tag="t2", name="t2_sb")
            nc.vector.tensor_add(t2_sb[:], t_sb[:], ps["h"][:])
            cal_sb = spool.tile([BT, D], F32, tag="cal", name="cal_sb")
            nc.scalar.activation(cal_sb[:], t2_sb[:], AF.Tanh)
            d_sb = spool.tile([BT, D], F32, tag="d", name="d_sb")
            nc.vector.tensor_sub(d_sb[:], cal_sb[:], h_sb[:])
            e_sb = spool.tile([BT, D], F32, tag="e", name="e_sb")
            nc.vector.scalar_tensor_tensor(e_sb[:], u_sb[:], att_sb, d_sb[:],
                                           ALU.mult, ALU.mult)
            o_sb = spool.tile([BT, D], F32, tag="o", name="o_sb")
            nc.vector.tensor_add(o_sb[:], h_sb[:], e_sb[:])
            nc.gpsimd.dma_start(out=out_d[row, :], in_=o_sb[:])

    with TileContext(nc) as tc:
        with (
            tc.tile_pool(name="const", bufs=1) as cp,
            tc.tile_pool(name="w", bufs=1) as wpool,
            tc.tile_pool(name="x", bufs=4) as xpool,
            tc.tile_pool(name="tpsum", bufs=2, space="PSUM") as tppool,
            tc.tile_pool(name="gpsum", bufs=1, space="PSUM") as gppool,
            tc.tile_pool(name="s", bufs=4) as spool,
        ):
            ident = cp.tile([128, 128], F32)
            masks.make_identity(nc, ident[:])
            ones = None
            if has_bias:
                ones = cp.tile([1, 128], F32)
                nc.vector.memset(ones[:], 1.0)
            if loop:
                hints = (mybir.EngineType.PE, mybir.EngineType.DVE,
                         mybir.EngineType.Activation, mybir.EngineType.SP)
                with tc.For_i(0, loop, 1, hint_engines=hints,
                              staggered_reset=staggered):
                    w_sb, b_sb = load_w(wpool)
                    body(w_sb, b_sb, ident, ones, xpool, tppool, gppool, spool)
            else:
                w_sb, b_sb = load_w(wpool)
                body(w_sb, b_sb, ident, ones, xpool, tppool, gppool, spool)

    nc.compile()
    return nc


def shard_inputs(inputs):
    in_maps = []
    for c in range(N_CORES):
        row = slice(c * BS, (c + 1) * BS)
        m = {
            "x": np.ascontiguousarray(inputs["input_x"][row], dtype=np.float32),
            "h": np.ascontiguousarray(inputs["input_h"][row], dtype=np.float32),
            "att": np.ascontiguousarray(inputs["attention_score"][row],
                                        dtype=np.float32),
        }
        for g in GATES:
            m[f"Wx_{g}"] = np.ascontiguousarray(inputs[f"Wx_{g}"], dtype=np.float32)
            m[f"Wh_{g}"] = np.ascontiguousarray(inputs[f"Wh_{g}"], dtype=np.float32)
            m[f"b_{g}"] = np.ascontiguousarray(inputs[f"b_{g}"], dtype=np.float32)
        in_maps.append(m)
    return in_maps


_cache = {}


def _get_program(has_bias: bool, loop: int = 0):
    key = (has_bias, loop)
    if key not in _cache:
        _cache[key] = build(has_bias, loop=loop)
    return _cache[key]


_exec_cache = {}

# DRAM-tensor name -> (key into the kernel() inputs dict, sharded-over-batch?)
_INPUT_MAP = {"x": ("input_x", True), "h": ("input_h", True),
              "att": ("attention_score", True)}
for _g in GATES:
    _INPUT_MAP[f"Wx_{_g}"] = (f"Wx_{_g}", False)
    _INPUT_MAP[f"Wh_{_g}"] = (f"Wh_{_g}", False)
    _INPUT_MAP[f"b_{_g}"] = (f"b_{_g}", False)


def _get_executable(has_bias: bool):
    """jit the bass program once per process; reuse across kernel() calls.

    Batch tensors (x/h/att) are sharded over the 8 cores; the weight matrices
    and biases are replicated (transferred once, not 8x)."""
    if has_bias in _exec_cache:
        return _exec_cache[has_bias]
    import jax
    from jax.sharding import Mesh, PartitionSpec, NamedSharding
    from jax.experimental.shard_map import shard_map
    from concourse import bass2jax

    nc = _get_program(has_bias)
    bass2jax.install_neuronx_cc_hook()
    partition_name = nc.partition_id_tensor.name if nc.partition_id_tensor else None
    in_names, out_names, out_avals = [], [], []
    for alloc in nc.m.functions[0].allocations:
        if not isinstance(alloc, mybir.MemoryLocationSet):
            continue
        name = alloc.memorylocations[0].name
        if alloc.kind == "ExternalInput":
            if name != partition_name:
                in_names.append(name)
        elif alloc.kind == "ExternalOutput":
            out_names.append(name)
            out_avals.append(jax.core.ShapedArray(
                tuple(alloc.tensor_shape), mybir.dt.np(alloc.dtype)))
    all_in_names = list(in_names) + out_names
    if partition_name is not None:
        all_in_names.append(partition_name)

    def _body(*args):
        operands = list(args)
        if partition_name is not None:
            operands.append(bass2jax.partition_id_tensor())
        return tuple(bass2jax._bass_exec_p.bind(
            *operands, out_avals=tuple(out_avals), in_names=tuple(all_in_names),
            out_names=tuple(out_names), lowering_input_output_aliases=(),
            sim_require_finite=True, sim_require_nnan=True, nc=nc))

    mesh = Mesh(np.asarray(jax.devices()[:N_CORES]), ("core",))
    in_specs = tuple(
        PartitionSpec("core") if _INPUT_MAP[nm][1] else PartitionSpec()
        for nm in in_names) + (PartitionSpec("core"),) * len(out_names)
    sharded = jax.jit(shard_map(
        _body, mesh=mesh, in_specs=in_specs,
        out_specs=(PartitionSpec("core"),) * len(out_names), check_rep=False))
    sh_batch = NamedSharding(mesh, PartitionSpec("core"))
    sh_repl = NamedSharding(mesh, PartitionSpec())
    zero_args = [jax.device_put(
        np.zeros((N_CORES * a.shape[0], *a.shape[1:]), a.dtype), sh_batch)
        for a in out_avals]
    entry = (sharded, sh_batch, sh_repl, in_names, out_names, zero_args, jax, {})
    _exec_cache[has_bias] = entry
    return entry


def kernel(**inputs) -> np.ndarray:
    inputs = {k: np.asarray(v) for k, v in inputs.items()}
    has_bias = any(np.any(inputs[f"b_{g}"]) for g in GATES)
    try:
        (sharded, sh_batch, sh_repl, in_names, out_names, zero_args, jax,
         dev_cache) = _get_executable(has_bias)
        args = []
        for nm in in_names:
            key, sharded_in = _INPUT_MAP[nm]
            arr = np.ascontiguousarray(inputs[key], dtype=np.float32)
            cached = dev_cache.get(nm)
            if cached is not None and np.array_equal(cached[0], arr):
                args.append(cached[1])
                continue
            dev = jax.device_put(arr, sh_batch if sharded_in else sh_repl)
            dev_cache[nm] = (arr.copy(), dev)
            args.append(dev)
        outs = sharded(*args, *zero_args)
        return np.asarray(outs[out_names.index("out")])
    except Exception:
        # fall back to the library path (and ride out transient hiccups)
        nc = _get_program(has_bias)
        in_maps = shard_inputs(inputs)
        res = run_bass_kernel_spmd(nc, in_maps, list(range(N_CORES)))
        return np.concatenate([res.results[c]["out"] for c in range(N_CORES)],
                              axis=0)

